# revision 23
# baseline (speedup 1.0000x reference)
"""Trainium2 Bass kernel for multi-head attention (B=2, S=2048, D=1024, H=16).

Sharding: 8 cores = 2 batches x 4 head-groups. Core c handles batch c//4 and
heads [4*(c%4), 4*(c%4)+4). Each core computes its 4 heads' Q/K/V projections
(column-sharded), attention, and a partial output projection over its 256
rows of Wo. Host sums the 4 partials per batch (tensor-parallel unshard).

Device-side layout choices:
  - Q/K kept transposed per head: QhT/KhT [hd, s] so logits are computed
    directly in [k, q] (transposed) orientation; softmax denominators come
    from a ones-column matmul on the PE; attn@V consumes exp weights with k
    on partitions, producing attnT [hd, q] which feeds the output projection
    as the stationary operand without any transposes anywhere.
  - All matmuls in bf16 (fp32 PSUM accumulation); softmax in fp32.
"""

import os
import sys

import numpy as np

sys.path.insert(0, "/opt/trn_rl_repo")

import ml_dtypes

B, S, D, H = 2, 2048, 1024, 16
HD = D // H          # 64 head dim
NCORES = 8
CPB = 4              # cores per batch
NHC = 4              # heads per core
COLS = NHC * HD      # 256 projection columns per core
NG = 2               # groups of 128 cols (head pairs)
QTS = 512            # q tile size
NQT = S // QTS       # 4
KTS = 128            # k tile size
NKT = S // KTS       # 16
NDC = D // 128       # 8 contraction chunks for projections
DOT = 512            # out-proj column tile
NDO = D // DOT       # 2
SCALE = 1.0 / float(np.sqrt(HD))

_PROGRAM = None
DEBUG = bool(int(os.environ.get("MHA_KERNEL_DEBUG", "0")))


def _build_program():
    import concourse.bass as bass
    import concourse.tile as tile
    from concourse import bacc
    import concourse.mybir as mybir

    f32 = mybir.dt.float32
    bf16 = mybir.dt.bfloat16
    AF = mybir.ActivationFunctionType
    PSUM = bass.MemorySpace.PSUM

    nc = bacc.Bacc("TRN2", target_bir_lowering=False, debug=False)

    qT_d = nc.dram_tensor("qT", [D, S], bf16, kind="ExternalInput")
    kT_d = nc.dram_tensor("kT", [D, S], bf16, kind="ExternalInput")
    vT_d = nc.dram_tensor("vT", [D, S], bf16, kind="ExternalInput")
    wq_d = nc.dram_tensor("wq", [D, COLS], bf16, kind="ExternalInput")
    wk_d = nc.dram_tensor("wk", [D, COLS], bf16, kind="ExternalInput")
    wv_d = nc.dram_tensor("wv", [D, COLS], bf16, kind="ExternalInput")
    wo_d = nc.dram_tensor("wo", [COLS, D], bf16, kind="ExternalInput")
    bqr_d = nc.dram_tensor("bqr", [128, NG], f32, kind="ExternalInput")
    bkr_d = nc.dram_tensor("bkr", [128, NG], f32, kind="ExternalInput")
    bvr_d = nc.dram_tensor("bvr", [128, COLS], f32, kind="ExternalInput")
    bor_d = nc.dram_tensor("bor", [128, D], f32, kind="ExternalInput")
    out_d = nc.dram_tensor("out", [S, D], f32, kind="ExternalOutput")
    if DEBUG:
        dbg_qhT = nc.dram_tensor("dbg_qhT", [128, NG, S], bf16, kind="ExternalOutput")
        dbg_khT = nc.dram_tensor("dbg_khT", [128, NG, S], bf16, kind="ExternalOutput")
        dbg_vh = nc.dram_tensor("dbg_vh", [128, NKT, COLS], bf16,
                                kind="ExternalOutput")
        dbg_exp = nc.dram_tensor("dbg_exp", [128, NHC, NKT, QTS], bf16,
                                 kind="ExternalOutput")
        dbg_r = nc.dram_tensor("dbg_r", [128, NG, QTS], f32, kind="ExternalOutput")
        dbg_av = nc.dram_tensor("dbg_av", [128, NG, QTS], f32, kind="ExternalOutput")

    with tile.TileContext(nc) as tc:
        with (
            tc.tile_pool(name="persist", bufs=1) as persist,
            tc.tile_pool(name="rpool", bufs=2) as rpool,
            tc.tile_pool(name="outstage", bufs=3) as outstage,
        ):
            # ---- persistent SBUF tiles ----
            QhT = persist.tile([128, NG, S], bf16)       # [p, grp, s]
            KhT = persist.tile([128, NG, S], bf16)
            Vh = persist.tile([128, NKT, COLS], bf16)    # [p, ktile, col]
            attnT = persist.tile([128, NG, S], bf16)
            wo_sb = persist.tile([128, NG, D], bf16)
            bqr_sb = persist.tile([128, NG], f32)
            bkr_sb = persist.tile([128, NG], f32)
            bv_bc = persist.tile([128, COLS], f32)
            bo_bc = persist.tile([128, D], f32)
            ones_sb = persist.tile([128, HD], bf16)

            nc.sync.dma_start(out=wo_sb[:],
                              in_=wo_d[:].rearrange("(c p) d -> p c d", p=128))
            nc.sync.dma_start(out=bqr_sb[:], in_=bqr_d[:])
            nc.sync.dma_start(out=bkr_sb[:], in_=bkr_d[:])
            nc.sync.dma_start(out=bv_bc[:], in_=bvr_d[:])
            nc.sync.dma_start(out=bo_bc[:], in_=bor_d[:])
            nc.vector.memset(ones_sb[:], 1.0)

            # ---- phase 1: Q/K projections ----
            inpV = tc.alloc_tile_pool(name="inpV", bufs=1)
            inpQK = tc.alloc_tile_pool(name="inpQK", bufs=1)
            projp = tc.alloc_tile_pool(name="projp", bufs=4, space=PSUM)

            qT_sb = inpQK.tile([128, NDC, S], bf16)
            kT_sb = inpQK.tile([128, NDC, S], bf16)
            wq_sb = inpQK.tile([128, NDC, COLS], bf16)
            wk_sb = inpQK.tile([128, NDC, COLS], bf16)
            vT_sb = inpV.tile([128, NDC, S], bf16)
            wv_sb = inpV.tile([128, NDC, COLS], bf16)

            nc.sync.dma_start(out=qT_sb[:],
                              in_=qT_d[:].rearrange("(c p) s -> p c s", p=128))
            nc.sync.dma_start(out=wq_sb[:],
                              in_=wq_d[:].rearrange("(c p) n -> p c n", p=128))
            nc.sync.dma_start(out=kT_sb[:],
                              in_=kT_d[:].rearrange("(c p) s -> p c s", p=128))
            nc.sync.dma_start(out=wk_sb[:],
                              in_=wk_d[:].rearrange("(c p) n -> p c n", p=128))
            nc.sync.dma_start(out=vT_sb[:],
                              in_=vT_d[:].rearrange("(c p) s -> p c s", p=128))
            nc.sync.dma_start(out=wv_sb[:],
                              in_=wv_d[:].rearrange("(c p) n -> p c n", p=128))

            # Q and K projections -> transposed head layout
            for dst, w_sb, x_sb, b_sb in (
                (QhT, wq_sb, qT_sb, bqr_sb),
                (KhT, wk_sb, kT_sb, bkr_sb),
            ):
                for g in range(NG):
                    for qt in range(NQT):
                        qk_ps = projp.tile([128, QTS], f32, tag="qk", name="qk_ps")
                        for dc in range(NDC):
                            nc.tensor.matmul(
                                qk_ps[:],
                                w_sb[:, dc, g * 128:(g + 1) * 128],
                                x_sb[:, dc, qt * QTS:(qt + 1) * QTS],
                                start=(dc == 0), stop=(dc == NDC - 1),
                            )
                        nc.vector.tensor_scalar_add(
                            dst[:, g, qt * QTS:(qt + 1) * QTS],
                            qk_ps[:], b_sb[:, g:g + 1])

            projp.release()
            inpQK.release()

            # ---- phase 2: V projection + attention + output projection ----
            with (
                tc.tile_pool(name="expp", bufs=1) as expp,
                tc.tile_pool(name="lpp", bufs=2, space=PSUM) as lpp,
                tc.tile_pool(name="avp", bufs=2, space=PSUM) as avp,
                tc.tile_pool(name="dnp", bufs=2, space=PSUM) as dnp,
            ):
                # V projection -> natural [s, col] layout (overlaps attention)
                for st in range(NKT):
                    v_ps = dnp.tile([128, COLS], f32, tag="dn", name="v_ps")
                    for dc in range(NDC):
                        nc.tensor.matmul(
                            v_ps[:],
                            vT_sb[:, dc, st * 128:(st + 1) * 128],
                            wv_sb[:, dc, :],
                            start=(dc == 0), stop=(dc == NDC - 1),
                        )
                    nc.vector.tensor_add(Vh[:, st, :], v_ps[:], bv_bc[:])

                if DEBUG:
                    nc.sync.dma_start(out=dbg_qhT[:], in_=QhT[:])
                    nc.sync.dma_start(out=dbg_khT[:], in_=KhT[:])
                    nc.sync.dma_start(out=dbg_vh[:], in_=Vh[:])

                def emit_avdn(ph, c, qt, expT, av_t, dn_t):
                    for h2 in range(2):
                        h = 2 * ph + h2
                        nc.tensor.matmul(
                            av_t[ph][h2 * 64:(h2 + 1) * 64, :],
                            Vh[:, c, h * HD:(h + 1) * HD],
                            expT[:, h, c, :],
                            start=(c == 0), stop=(c == NKT - 1),
                            tile_position=(0, h2 * 64),
                            skip_group_check=True,
                        )
                    for h2 in range(2):
                        h = 2 * ph + h2
                        nc.tensor.matmul(
                            dn_t[ph][h2 * 64:(h2 + 1) * 64, :],
                            ones_sb[:, :],
                            expT[:, h, c, :],
                            start=(c == 0), stop=(c == NKT - 1),
                            tile_position=(0, h2 * 64),
                            skip_group_check=True,
                        )

                for qt in range(NQT):
                    q0 = qt * QTS
                    expT = expp.tile([128, NHC, NKT, QTS], bf16, tag="expT",
                                     name="expT")
                    av_t = []
                    dn_t = []
                    for ph in range(NG):
                        av = avp.tile([128, QTS], f32, tag="av",
                                      name=f"av{ph}")
                        av_t.append(av)
                        dn = dnp.tile([128, QTS], f32, tag="dn",
                                      name=f"dn{ph}")
                        dn_t.append(dn)

                    # logits + exp per (pair, ktile); attn@V/denoms delayed one
                    # ktile so PE always has ready work while ACT runs exp
                    for c in range(NKT):
                        for ph in range(NG):
                            lp = lpp.tile([128, 2, QTS], f32, tag="Lp", name="lp")
                            for h2 in range(2):
                                pb = h2 * 64
                                nc.tensor.matmul(
                                    lp[:, h2, :],
                                    KhT[pb:pb + 64, ph, c * 128:(c + 1) * 128],
                                    QhT[pb:pb + 64, ph, q0:q0 + QTS],
                                    start=True, stop=True,
                                    tile_position=(pb, 0),
                                )
                            nc.scalar.activation(
                                expT[:, 2 * ph:2 * ph + 2, c, :],
                                lp[:],
                                AF.Exp, scale=SCALE,
                            )
                            if c > 0:
                                emit_avdn(ph, c - 1, qt, expT, av_t, dn_t)
                    for ph in range(NG):
                        emit_avdn(ph, NKT - 1, qt, expT, av_t, dn_t)

                    if DEBUG and qt == 0:
                        nc.sync.dma_start(out=dbg_exp[:], in_=expT[:])
                        for ph in range(NG):
                            dbg_st = outstage.tile([128, QTS], f32, tag="st",
                                                   name="dbg_st")
                            nc.vector.tensor_copy(dbg_st[:], av_t[ph][:])
                            nc.sync.dma_start(out=dbg_av[:, ph, :], in_=dbg_st[:])

                    # normalize: attnT = av / denom (denoms are row-replicated)
                    for ph in range(NG):
                        rb_t = rpool.tile([128, QTS], f32, tag="rb", bufs=4,
                                          name="rb_t")
                        nc.vector.reciprocal(rb_t[:], dn_t[ph][:])
                        if DEBUG and qt == 0:
                            nc.sync.dma_start(out=dbg_r[:, ph, :], in_=rb_t[:])
                        nc.vector.tensor_mul(attnT[:, ph, q0:q0 + QTS],
                                             av_t[ph][:], rb_t[:])

                    # output projection for this q tile (partial over 256 rows)
                    for qs in range(QTS // 128):
                        r0 = q0 + qs * 128
                        for do in range(NDO):
                            op_ps = dnp.tile([128, DOT], f32, tag="dn",
                                             name="op_ps")
                            for ch in range(NG):
                                nc.tensor.matmul(
                                    op_ps[:],
                                    attnT[:, ch, r0:r0 + 128],
                                    wo_sb[:, ch, do * DOT:(do + 1) * DOT],
                                    start=(ch == 0), stop=(ch == NG - 1),
                                )
                            st_t = outstage.tile([128, DOT], f32, tag="st",
                                                 name="st_t")
                            nc.vector.tensor_add(st_t[:], op_ps[:],
                                                 bo_bc[:, do * DOT:(do + 1) * DOT])
                            nc.sync.dma_start(
                                out=out_d[r0:r0 + 128, do * DOT:(do + 1) * DOT],
                                in_=st_t[:])
            inpV.release()

    nc.compile()
    return nc


def _get_program():
    global _PROGRAM
    if _PROGRAM is None:
        _PROGRAM = _build_program()
    return _PROGRAM


def make_in_maps(q, k, v, Wq, Wk, Wv, Wo, bq, bk, bv, bo):
    bf = ml_dtypes.bfloat16
    q = np.asarray(q, np.float32)
    k = np.asarray(k, np.float32)
    v = np.asarray(v, np.float32)
    Wq = np.asarray(Wq, np.float32)
    Wk = np.asarray(Wk, np.float32)
    Wv = np.asarray(Wv, np.float32)
    Wo = np.asarray(Wo, np.float32)
    bq = np.asarray(bq, np.float32)
    bk = np.asarray(bk, np.float32)
    bv = np.asarray(bv, np.float32)
    bo = np.asarray(bo, np.float32)

    qT = [np.ascontiguousarray(q[b].T).astype(bf) for b in range(B)]
    kT = [np.ascontiguousarray(k[b].T).astype(bf) for b in range(B)]
    vT = [np.ascontiguousarray(v[b].T).astype(bf) for b in range(B)]

    in_maps = []
    for c in range(NCORES):
        b, g = divmod(c, CPB)
        cs = slice(g * COLS, (g + 1) * COLS)
        in_maps.append({
            "qT": qT[b],
            "kT": kT[b],
            "vT": vT[b],
            "wq": np.ascontiguousarray(Wq[:, cs]).astype(bf),
            "wk": np.ascontiguousarray(Wk[:, cs]).astype(bf),
            "wv": np.ascontiguousarray(Wv[:, cs]).astype(bf),
            "wo": np.ascontiguousarray(Wo[cs, :]).astype(bf),
            "bqr": np.ascontiguousarray(bq[cs].reshape(NG, 128).T),
            "bkr": np.ascontiguousarray(bk[cs].reshape(NG, 128).T),
            "bvr": np.ascontiguousarray(
                np.broadcast_to(bv[cs].reshape(1, COLS), (128, COLS))),
            "bor": np.ascontiguousarray(np.broadcast_to(
                (bo if g == 0 else np.zeros_like(bo)).reshape(1, D), (128, D))),
        })
    return in_maps


def combine_outputs(results):
    out = np.zeros((B, S, D), np.float32)
    for c in range(NCORES):
        out[c // CPB] += results[c]["out"]
    return out


def kernel(q, k, v, Wq, Wk, Wv, Wo, bq, bk, bv, bo):
    from concourse.bass_utils import run_bass_kernel_spmd

    nc = _get_program()
    in_maps = make_in_maps(q, k, v, Wq, Wk, Wv, Wo, bq, bk, bv, bo)
    res = run_bass_kernel_spmd(nc, in_maps, list(range(NCORES)))
    return combine_outputs(res.results)


# revision 24
# speedup vs baseline: 14.4352x; 14.4352x over previous
"""Trainium2 Bass kernel for multi-head attention (B=2, S=2048, D=1024, H=16).

Sharding: 8 cores = 2 batches x 4 head-groups. Core c handles batch c//4 and
heads [4*(c%4), 4*(c%4)+4). Each core computes its 4 heads' Q/K/V projections
(column-sharded), attention, and a partial output projection over its 256
rows of Wo. Host sums the 4 partials per batch (tensor-parallel unshard).

Device-side layout choices:
  - Q/K kept transposed per head: QhT/KhT [hd, s] so logits are computed
    directly in [k, q] (transposed) orientation; softmax denominators come
    from a row-replicated ones-matrix matmul on the PE; attn@V consumes exp
    weights with k on partitions, producing attnT [hd, q] which feeds the
    output projection as the stationary operand without any transposes.
  - All matmuls in bf16 (fp32 PSUM accumulation); softmax in fp32.
  - Head pairs are packed into single PE slots via tile_position row/col
    tiling (concurrent 64-wide matmuls).
"""

import os
import sys

import numpy as np

sys.path.insert(0, "/opt/trn_rl_repo")

import ml_dtypes

B, S, D, H = 2, 2048, 1024, 16
HD = D // H          # 64 head dim
NCORES = 8
CPB = 4              # cores per batch
NHC = 4              # heads per core
COLS = NHC * HD      # 256 projection columns per core
NG = 2               # groups of 128 cols (head pairs)
QTS = 512            # q tile size
NQT = S // QTS       # 4
KTS = 128            # k tile size
NKT = S // KTS       # 16
NDC = D // 128       # 8 contraction chunks for projections
DOT = 512            # out-proj column tile
NDO = D // DOT       # 2
SCALE = 1.0 / float(np.sqrt(HD))

_PROGRAMS = {}
DEBUG = bool(int(os.environ.get("MHA_KERNEL_DEBUG", "0")))


def _build_program(loopn=1):
    import concourse.bass as bass
    import concourse.tile as tile
    from concourse import bacc
    import concourse.mybir as mybir

    f32 = mybir.dt.float32
    bf16 = mybir.dt.bfloat16
    AF = mybir.ActivationFunctionType
    PSUM = bass.MemorySpace.PSUM
    debug = DEBUG and loopn == 1

    nc = bacc.Bacc("TRN2", target_bir_lowering=False, debug=False)

    qT_d = nc.dram_tensor("qT", [D, S], bf16, kind="ExternalInput")
    kT_d = nc.dram_tensor("kT", [D, S], bf16, kind="ExternalInput")
    vT_d = nc.dram_tensor("vT", [D, S], bf16, kind="ExternalInput")
    wq_d = nc.dram_tensor("wq", [D, COLS], bf16, kind="ExternalInput")
    wk_d = nc.dram_tensor("wk", [D, COLS], bf16, kind="ExternalInput")
    wv_d = nc.dram_tensor("wv", [D, COLS], bf16, kind="ExternalInput")
    wo_d = nc.dram_tensor("wo", [COLS, D], bf16, kind="ExternalInput")
    bqr_d = nc.dram_tensor("bqr", [128, NG], f32, kind="ExternalInput")
    bkr_d = nc.dram_tensor("bkr", [128, NG], f32, kind="ExternalInput")
    bvr_d = nc.dram_tensor("bvr", [128, COLS], f32, kind="ExternalInput")
    bor_d = nc.dram_tensor("bor", [128, D], f32, kind="ExternalInput")
    out_d = nc.dram_tensor("out", [S, D], f32, kind="ExternalOutput")
    if debug:
        dbg_qhT = nc.dram_tensor("dbg_qhT", [128, NG, S], bf16, kind="ExternalOutput")
        dbg_khT = nc.dram_tensor("dbg_khT", [128, NG, S], bf16, kind="ExternalOutput")
        dbg_vh = nc.dram_tensor("dbg_vh", [128, NKT, COLS], bf16,
                                kind="ExternalOutput")
        dbg_exp = nc.dram_tensor("dbg_exp", [128, NHC, NKT, QTS], bf16,
                                 kind="ExternalOutput")
        dbg_r = nc.dram_tensor("dbg_r", [128, NG, QTS], f32, kind="ExternalOutput")
        dbg_av = nc.dram_tensor("dbg_av", [128, NG, QTS], f32, kind="ExternalOutput")

    with tile.TileContext(nc) as tc:
        with (
            tc.tile_pool(name="persist", bufs=1) as persist,
            tc.tile_pool(name="wpool", bufs=1) as wpool,
            tc.tile_pool(name="xstream", bufs=2) as xstream,
            tc.tile_pool(name="rpool", bufs=4) as rpool,
            tc.tile_pool(name="outstage", bufs=3) as outstage,
            tc.tile_pool(name="expp", bufs=1) as expp,
            tc.tile_pool(name="lpp", bufs=2, space=PSUM) as lpp,
            tc.tile_pool(name="avp", bufs=2, space=PSUM) as avp,
            tc.tile_pool(name="dnp", bufs=2, space=PSUM) as dnp,
        ):
            # ---- persistent SBUF tiles ----
            QhT = persist.tile([128, NG, S], bf16)       # [p, grp, s]
            KhT = persist.tile([128, NG, S], bf16)
            Vh = persist.tile([128, NKT, COLS], bf16)    # [p, ktile, col]
            attnT = persist.tile([128, NG, S], bf16)
            wo_sb = persist.tile([128, NG, D], bf16)
            bqr_sb = persist.tile([128, NG], f32)
            bkr_sb = persist.tile([128, NG], f32)
            bv_bc = persist.tile([128, COLS], f32)
            bo_bc = persist.tile([128, D], f32)
            ones_sb = persist.tile([128, HD], bf16)

            nc.vector.memset(ones_sb[:], 1.0)
            nc.sync.dma_start(out=bqr_sb[:], in_=bqr_d[:])
            nc.sync.dma_start(out=bkr_sb[:], in_=bkr_d[:])
            nc.sync.dma_start(out=bv_bc[:], in_=bvr_d[:])
            nc.sync.dma_start(out=bo_bc[:], in_=bor_d[:])

            def body(_iv=None):
                wq_sb = wpool.tile([128, NDC, COLS], bf16, tag="wq", name="wq_sb")
                wk_sb = wpool.tile([128, NDC, COLS], bf16, tag="wk", name="wk_sb")
                wv_sb = wpool.tile([128, NDC, COLS], bf16, tag="wv", name="wv_sb")
                nc.sync.dma_start(out=wq_sb[:],
                                  in_=wq_d[:].rearrange("(c p) n -> p c n", p=128))
                nc.sync.dma_start(out=wk_sb[:],
                                  in_=wk_d[:].rearrange("(c p) n -> p c n", p=128))
                nc.sync.dma_start(out=wv_sb[:],
                                  in_=wv_d[:].rearrange("(c p) n -> p c n", p=128))
                wo_l = wpool.tile([128, NG, D], bf16, tag="wo", name="wo_l")
                nc.sync.dma_start(out=wo_l[:],
                                  in_=wo_d[:].rearrange("(c p) d -> p c d", p=128))
                nc.vector.tensor_copy(wo_sb[:], wo_l[:])

                # Q and K projections -> transposed head layout, streamed by
                # q tile so compute starts as soon as the first slice lands
                qT_r = qT_d[:].rearrange("(c p) (t n) -> p c t n", p=128, n=QTS)
                kT_r = kT_d[:].rearrange("(c p) (t n) -> p c t n", p=128, n=QTS)
                for dst, w_sb, x_r, b_sb, xtag in (
                    (QhT, wq_sb, qT_r, bqr_sb, "qx"),
                    (KhT, wk_sb, kT_r, bkr_sb, "kx"),
                ):
                    for qt in range(NQT):
                        x_sb = xstream.tile([128, NDC, QTS], bf16, tag=xtag,
                                            name="x_sb")
                        nc.sync.dma_start(out=x_sb[:], in_=x_r[:, :, qt, :])
                        for g in range(NG):
                            ps = avp.tile([128, QTS], f32, tag="av", name="qk_ps")
                            for dc in range(NDC):
                                nc.tensor.matmul(
                                    ps[:],
                                    w_sb[:, dc, g * 128:(g + 1) * 128],
                                    x_sb[:, dc, :],
                                    start=(dc == 0), stop=(dc == NDC - 1),
                                )
                            nc.vector.tensor_scalar_add(
                                dst[:, g, qt * QTS:(qt + 1) * QTS],
                                ps[:], b_sb[:, g:g + 1])

                # V projection -> natural [s, col] layout (overlaps attention)
                vT_r = vT_d[:].rearrange("(c p) (t n) -> p c t n", p=128, n=QTS)
                for vt in range(NQT):
                    v_sb = xstream.tile([128, NDC, QTS], bf16, tag="vx",
                                        name="v_sb")
                    nc.sync.dma_start(out=v_sb[:], in_=vT_r[:, :, vt, :])
                    for sst in range(QTS // 128):
                        st = vt * 4 + sst
                        v_ps = dnp.tile([128, COLS], f32, tag="dn", name="v_ps")
                        for dc in range(NDC):
                            nc.tensor.matmul(
                                v_ps[:],
                                v_sb[:, dc, sst * 128:(sst + 1) * 128],
                                wv_sb[:, dc, :],
                                start=(dc == 0), stop=(dc == NDC - 1),
                            )
                        nc.vector.tensor_add(Vh[:, st, :], v_ps[:], bv_bc[:])

                if debug:
                    nc.sync.dma_start(out=dbg_qhT[:], in_=QhT[:])
                    nc.sync.dma_start(out=dbg_khT[:], in_=KhT[:])
                    nc.sync.dma_start(out=dbg_vh[:], in_=Vh[:])

                def emit_avdn(ph, c, expT, av_t, dn_t):
                    for h2 in range(2):
                        h = 2 * ph + h2
                        nc.tensor.matmul(
                            av_t[ph][h2 * 64:(h2 + 1) * 64, :],
                            Vh[:, c, h * HD:(h + 1) * HD],
                            expT[:, h, c, :],
                            start=(c == 0), stop=(c == NKT - 1),
                            tile_position=(0, h2 * 64),
                            skip_group_check=True,
                        )
                    for h2 in range(2):
                        h = 2 * ph + h2
                        nc.tensor.matmul(
                            dn_t[ph][h2 * 64:(h2 + 1) * 64, :],
                            ones_sb[:, :],
                            expT[:, h, c, :],
                            start=(c == 0), stop=(c == NKT - 1),
                            tile_position=(0, h2 * 64),
                            skip_group_check=True,
                        )

                for qt in range(NQT):
                    q0 = qt * QTS
                    expT = expp.tile([128, NHC, NKT, QTS], bf16, tag="expT",
                                     name="expT")
                    av_t = []
                    dn_t = []
                    for ph in range(NG):
                        av = avp.tile([128, QTS], f32, tag="av", name=f"av{ph}")
                        av_t.append(av)
                        dn = dnp.tile([128, QTS], f32, tag="dn", name=f"dn{ph}")
                        dn_t.append(dn)

                    # logits + exp per (pair, ktile); attn@V/denoms delayed one
                    # ktile so PE always has ready work while ACT runs exp
                    for c in range(NKT):
                        for ph in range(NG):
                            lp = lpp.tile([128, 2, QTS], f32, tag="Lp", name="lp")
                            for h2 in range(2):
                                pb = h2 * 64
                                nc.tensor.matmul(
                                    lp[:, h2, :],
                                    KhT[pb:pb + 64, ph, c * 128:(c + 1) * 128],
                                    QhT[pb:pb + 64, ph, q0:q0 + QTS],
                                    start=True, stop=True,
                                    tile_position=(pb, 0),
                                )
                            nc.scalar.activation(
                                expT[:, 2 * ph:2 * ph + 2, c, :],
                                lp[:],
                                AF.Exp, scale=SCALE,
                            )
                            if c > 0:
                                emit_avdn(ph, c - 1, expT, av_t, dn_t)
                    for ph in range(NG):
                        emit_avdn(ph, NKT - 1, expT, av_t, dn_t)

                    if debug and qt == 0:
                        nc.sync.dma_start(out=dbg_exp[:], in_=expT[:])
                        for ph in range(NG):
                            dbg_st = outstage.tile([128, QTS], f32, tag="st",
                                                   name="dbg_st")
                            nc.vector.tensor_copy(dbg_st[:], av_t[ph][:])
                            nc.sync.dma_start(out=dbg_av[:, ph, :], in_=dbg_st[:])

                    # normalize: attnT = av / denom (denoms are row-replicated)
                    for ph in range(NG):
                        rb_t = rpool.tile([128, QTS], f32, tag="rb", name="rb_t")
                        nc.vector.reciprocal(rb_t[:], dn_t[ph][:])
                        if debug and qt == 0:
                            nc.sync.dma_start(out=dbg_r[:, ph, :], in_=rb_t[:])
                        nc.vector.tensor_mul(attnT[:, ph, q0:q0 + QTS],
                                             av_t[ph][:], rb_t[:])

                    # output projection for this q tile (partial, 256 rows)
                    for qs in range(QTS // 128):
                        r0 = q0 + qs * 128
                        for do in range(NDO):
                            op_ps = dnp.tile([128, DOT], f32, tag="dn",
                                             name="op_ps")
                            for ch in range(NG):
                                nc.tensor.matmul(
                                    op_ps[:],
                                    attnT[:, ch, r0:r0 + 128],
                                    wo_sb[:, ch, do * DOT:(do + 1) * DOT],
                                    start=(ch == 0), stop=(ch == NG - 1),
                                )
                            st_t = outstage.tile([128, DOT], f32, tag="st",
                                                 name="st_t")
                            nc.vector.tensor_add(st_t[:], op_ps[:],
                                                 bo_bc[:, do * DOT:(do + 1) * DOT])
                            nc.sync.dma_start(
                                out=out_d[r0:r0 + 128, do * DOT:(do + 1) * DOT],
                                in_=st_t[:])

            if loopn == 1:
                body()
            else:
                with tc.For_i(0, loopn, 1) as iv:
                    body(iv)

    nc.compile()
    return nc


def _get_program(loopn=1):
    if loopn not in _PROGRAMS:
        _PROGRAMS[loopn] = _build_program(loopn)
    return _PROGRAMS[loopn]


def make_in_maps(q, k, v, Wq, Wk, Wv, Wo, bq, bk, bv, bo):
    bf = ml_dtypes.bfloat16
    q = np.asarray(q, np.float32)
    k = np.asarray(k, np.float32)
    v = np.asarray(v, np.float32)
    Wq = np.asarray(Wq, np.float32)
    Wk = np.asarray(Wk, np.float32)
    Wv = np.asarray(Wv, np.float32)
    Wo = np.asarray(Wo, np.float32)
    bq = np.asarray(bq, np.float32)
    bk = np.asarray(bk, np.float32)
    bv = np.asarray(bv, np.float32)
    bo = np.asarray(bo, np.float32)

    qT = [np.ascontiguousarray(q[b].T).astype(bf) for b in range(B)]
    kT = [np.ascontiguousarray(k[b].T).astype(bf) for b in range(B)]
    vT = [np.ascontiguousarray(v[b].T).astype(bf) for b in range(B)]

    in_maps = []
    for c in range(NCORES):
        b, g = divmod(c, CPB)
        cs = slice(g * COLS, (g + 1) * COLS)
        in_maps.append({
            "qT": qT[b],
            "kT": kT[b],
            "vT": vT[b],
            "wq": np.ascontiguousarray(Wq[:, cs]).astype(bf),
            "wk": np.ascontiguousarray(Wk[:, cs]).astype(bf),
            "wv": np.ascontiguousarray(Wv[:, cs]).astype(bf),
            "wo": np.ascontiguousarray(Wo[cs, :]).astype(bf),
            "bqr": np.ascontiguousarray(bq[cs].reshape(NG, 128).T),
            "bkr": np.ascontiguousarray(bk[cs].reshape(NG, 128).T),
            "bvr": np.ascontiguousarray(
                np.broadcast_to(bv[cs].reshape(1, COLS), (128, COLS))),
            "bor": np.ascontiguousarray(np.broadcast_to(
                (bo if g == 0 else np.zeros_like(bo)).reshape(1, D), (128, D))),
        })
    return in_maps


def combine_outputs(results):
    out = np.zeros((B, S, D), np.float32)
    for c in range(NCORES):
        out[c // CPB] += results[c]["out"]
    return out


def kernel(q, k, v, Wq, Wk, Wv, Wo, bq, bk, bv, bo):
    from concourse.bass_utils import run_bass_kernel_spmd

    nc = _get_program()
    in_maps = make_in_maps(q, k, v, Wq, Wk, Wv, Wo, bq, bk, bv, bo)
    res = run_bass_kernel_spmd(nc, in_maps, list(range(NCORES)))
    return combine_outputs(res.results)
